# revision 1
# baseline (speedup 1.0000x reference)
"""CMAttention Trainium2 kernel (Bass/Tile), data-parallel over batch on 8 cores.

Reference computation (per batch b, per head h, d=64, n=1024):
  q = inp @ Wq.T + bq                    (split heads)
  k = [ctx @ Wk.T + bk ; sqrt(64)*mk]    ctx = [x;y], 2051 keys
  cross = softmax(q k^T / 8) @ Wf.T + bf          [n, 1027]
  Sk = [inp @ Wk.T + bk ; sqrt(64)*smk]  1027 self keys
  self = softmax(q Sk^T / 8)                       [n, 1027]
  Sv = [inp @ Wv.T + bv ; sqrt(3)*smv]             [1027, 64]
  out_h = (cross + self) @ Sv
  out = concat_h(out_h) @ Wo.T + bo

Kernel strategy (everything transposed: features/keys on partitions):
  - All activations/weights pre-transposed and cast to bf16 on host; bf16
    matmuls with fp32 PSUM accumulation throughout (norm rel err ~6e-3).
  - dots^T[j, i] computed directly via matmul(lhsT=KT_h, rhs=QT_h) (K=64),
    both 512-wide i-chunks into one [128, 1024] PSUM tile -> a single
    Exp per j-tile (ACT per-instruction overhead amortized; ACT is the
    co-bottleneck engine with PE).
  - Unnormalized softmax: E = exp(dots*scale) (bf16), no max subtraction
    (arguments are O(+-6), safely in fp32/bf16 exp range).
  - KEY TRICK: the reference's huge fuse matmul softmax(QK^T) @ Wf.T
    (276 of 345 total GFLOPs) is never materialized.  Since its only
    consumer is the final "@ Sv" contraction, associativity gives
      (E @ Wf.T) @ Sv = E @ (Wf.T @ Sv) = E @ G,   G = Wf.T @ Sv
    where G is [2051, 64] per head - computed once per head for ~16x
    less work.  G carries a ones column so U1's PSUM row 64 = Z (the
    softmax denominator); normalization moves past both matmuls and is
    applied as a per-row reciprocal broadcast at the [64, 1024] stage.
  - U1^T = G^T-contract-E (cross branch), U2^T = Sv^T-contract-E2 (self
    branch, ones column -> Z2).  contrib = U1/Z + U2/Z2 via fp32
    reciprocal + gpsimd partition_broadcast + DVE multiplies.
  - Head 0's dots+exp are emitted before the Sv/G phase so the ACT
    engine starts early and stays saturated.
  - Final projection contracts out^T with Wo^T; bias row (bo + bf@Sv@Wo^T)
    added via a K=1 ones matmul.  All biases (zero in this problem's
    setup_inputs, but implemented faithfully and tested nonzero) ride
    per-partition tensor_scalar adds or host-prepared broadcast tiles.
"""
import numpy as np
import ml_dtypes
from contextlib import ExitStack

import concourse.bass as bass
import concourse.tile as tile
from concourse import bacc, mybir
from concourse import bass_utils

F32 = mybir.dt.float32
BF16 = mybir.dt.bfloat16
bfnp = ml_dtypes.bfloat16
AF = mybir.ActivationFunctionType
ALU = mybir.AluOpType

B = 8
N = 1024
DIM = 512
HEADS = 8
DH = 64
M = 3
SCALE = 0.125
NK = 2 * N + M          # 2051 cross keys
NF = N + M              # 1027 fuse outputs / self keys
NKT = 17                # cross j-tiles: 16 full + [3 mem + 1 bias row]
NFT = 9                 # self j2-tiles: 8 full + 3 mem
CH = 2                  # i-chunks of 512
CW = 512


def build_kernel(tc):
    nc = tc.nc
    d = {}
    def din(name, shape, dt=BF16):
        d[name] = nc.dram_tensor(name, shape, dt, kind="ExternalInput").ap()
    din("inpT", [DIM, N])
    din("ctxT", [DIM, 2 * N])
    din("wqT", [DIM, DIM])
    din("wkT", [DIM, DIM])
    din("wvT", [DIM, DIM])
    din("woT", [DIM, DIM])
    din("wfN", [NF, NK])         # Wf natural [1027, 2051]
    din("memKT", [DIM, M])
    din("memSkT", [DIM, M])
    din("svmemB", [M, 520])      # rows [Sv1024..26], ones col
    din("b_sv", [128, DIM])      # bkv_v broadcast tile
    din("bq", [DIM, 1], F32)
    din("bk", [DIM, 1], F32)
    din("bfcol", [NF, 1])        # bf
    din("borow", [1, DIM])
    out_d = nc.dram_tensor("out", [N, DIM], F32, kind="ExternalOutput").ap()

    ctx = ExitStack()
    with ctx:
        pers = ctx.enter_context(tc.tile_pool(name="pers", bufs=1))
        ppd = ctx.enter_context(tc.tile_pool(name="ppd", bufs=2, space="PSUM"))
        ppu = ctx.enter_context(tc.tile_pool(name="ppu", bufs=1, space="PSUM"))

        # ---------------- persistent SBUF ----------------
        woT = [pers.tile([128, DIM], BF16, tag=f"woT{t}", name=f"woT{t}") for t in range(4)]
        Gst = [pers.tile([128 if t < 16 else M, 520], BF16, tag=f"Gst{t}", name=f"Gst{t}")
               for t in range(NKT)]
        QT = [pers.tile([128, N], BF16, tag=f"QT{t}", name=f"QT{t}") for t in range(4)]
        KT = [pers.tile([128, NK], BF16, tag=f"KT{t}", name=f"KT{t}") for t in range(4)]
        SkT = [pers.tile([128, NF], BF16, tag=f"SkT{t}", name=f"SkT{t}") for t in range(4)]
        Sv = [pers.tile([128, 520], BF16, tag=f"Sv{t}", name=f"Sv{t}") for t in range(8)]
        svB = pers.tile([M, 520], BF16, tag="svB", name="svB")
        outT = [pers.tile([128, N], BF16, tag=f"outT{t}", name=f"outT{t}") for t in range(4)]
        bsv = pers.tile([128, DIM], BF16, tag="bsv", name="bsv")
        bq = [pers.tile([128, 1], F32, tag=f"bq{t}", name=f"bq{t}") for t in range(4)]
        bk = [pers.tile([128, 1], F32, tag=f"bk{t}", name=f"bk{t}") for t in range(4)]
        bfc = [pers.tile([128 if t < 8 else M, 1], BF16, tag=f"bfc{t}", name=f"bfc{t}")
               for t in range(NFT)]
        borow = pers.tile([1, DIM], BF16, tag="borow", name="borow")
        ones128 = pers.tile([1, 128], BF16, tag="ones128", name="ones128")
        wT = [pers.tile([128, 1], BF16, tag=f"wT{t}", name=f"wT{t}") for t in range(4)]
        wob = pers.tile([1, DIM], BF16, tag="wob", name="wob")

        # ---------------- input DMAs (persistent) ----------------
        for t in range(4):
            nc.sync.dma_start(woT[t][:], d["woT"][128 * t:128 * t + 128, :])
            nc.sync.dma_start(bq[t][:], d["bq"][128 * t:128 * t + 128, :])
            nc.sync.dma_start(bk[t][:], d["bk"][128 * t:128 * t + 128, :])
            nc.sync.dma_start(KT[t][:, 2 * N:], d["memKT"][128 * t:128 * t + 128, :])
            nc.sync.dma_start(SkT[t][:, N:], d["memSkT"][128 * t:128 * t + 128, :])
        for t in range(NFT):
            r0 = 128 * t
            r1 = min(r0 + 128, NF)
            nc.sync.dma_start(bfc[t][:], d["bfcol"][r0:r1, :])
        nc.sync.dma_start(svB[:], d["svmemB"][:])
        nc.sync.dma_start(bsv[:], d["b_sv"][:])
        nc.sync.dma_start(borow[:], d["borow"][:])
        nc.vector.memset(ones128[:], 1.0)

        work = ctx.enter_context(tc.tile_pool(name="work", bufs=2))
        epool = ctx.enter_context(tc.tile_pool(name="epool", bufs=1))
        fpool = ctx.enter_context(tc.tile_pool(name="fpool", bufs=2))

        def dots_phase(h):
            """dots^T + exp for one head; both i-chunks in one [128,1024] psum."""
            ht, sd = divmod(h, 2)
            hp = 64 * sd
            E, E2 = {}, {}
            for src, nt, full, tagc, store in ((KT, NKT, 16, "E", E),
                                               (SkT, NFT, 8, "F", E2)):
                for t in range(nt):
                    mw = 128 if t < full else M
                    ps = ppd.tile([128, 2 * CW], F32, tag="pdE", name="pdE")
                    for c in range(CH):
                        nc.tensor.matmul(ps[0:mw, CW * c:CW * c + CW],
                                         src[ht][hp:hp + 64, 128 * t:128 * t + mw],
                                         QT[ht][hp:hp + 64, CW * c:CW * c + CW],
                                         start=True, stop=True)
                    pool = fpool if (tagc == "F" and t < 8) else epool
                    e = pool.tile([128 if t < full else M, 2 * CW], BF16,
                                  tag=f"{tagc}{t}", name=f"{tagc}{t}")
                    store[t] = e
                    nc.scalar.activation(e[0:mw, :], ps[0:mw, :], AF.Exp,
                                         scale=SCALE)
            return E, E2

        def u_phase(h, E, E2):
            ht, sd = divmod(h, 2)
            hp = 64 * sd
            # U1^T = G^T-contract-E; ones col -> row 64 = Z
            pu1 = ppu.tile([65, 2 * CW], F32, tag="pu1", name="pu1")
            for t in range(NKT):
                kw = 128 if t < 16 else M
                for c in range(CH):
                    cs = slice(CW * c, CW * c + CW)
                    nc.tensor.matmul(pu1[:, cs],
                                     Gst[t][0:kw, 65 * h:65 * h + 65],
                                     E[t][0:kw, cs],
                                     start=(t == 0), stop=(t == NKT - 1))
            rzb = work.tile([64, 2 * CW], F32, tag="rzb", name="rzb")
            nc.vector.reciprocal(rzb[0:1, :], pu1[64:65, :])
            nc.gpsimd.partition_broadcast(rzb[:], rzb[0:1, :])
            # U2^T with ones col -> row 64 = Z2
            pu2 = ppu.tile([65, 2 * CW], F32, tag="pu2", name="pu2")
            for t in range(NFT):
                kw = 128 if t < 8 else M
                lhs = (Sv[t] if t < 8 else svB)[0:kw, 65 * h:65 * h + 65]
                for c in range(CH):
                    cs = slice(CW * c, CW * c + CW)
                    nc.tensor.matmul(pu2[:, cs], lhs, E2[t][0:kw, cs],
                                     start=(t == 0), stop=(t == NFT - 1))
            rz2b = work.tile([64, 2 * CW], F32, tag="rz2b", name="rz2b")
            nc.vector.reciprocal(rz2b[0:1, :], pu2[64:65, :])
            nc.gpsimd.partition_broadcast(rz2b[:], rz2b[0:1, :])
            tmp = work.tile([64, 2 * CW], BF16, tag="tmp", name="tmp")
            nc.vector.tensor_tensor(tmp[:], pu2[0:64, :], rz2b[:], ALU.mult)
            tmp1 = work.tile([64, 2 * CW], BF16, tag="tmp1", name="tmp1")
            nc.vector.tensor_tensor(tmp1[:], pu1[0:64, :], rzb[:], ALU.mult)
            nc.vector.tensor_tensor(outT[ht][hp:hp + 64, :], tmp1[:],
                                    tmp[:], ALU.add)

        # ---------------- projections (scoped pool, released after) --------
        with tc.tile_pool(name="projp", bufs=1) as projp:
            inpT = [projp.tile([128, N], BF16, tag=f"inpT{t}", name=f"inpT{t}")
                    for t in range(4)]
            ctxT = [projp.tile([128, 2 * N], BF16, tag=f"ctxT{t}", name=f"ctxT{t}")
                    for t in range(4)]
            wqT = [projp.tile([128, DIM], BF16, tag=f"wqT{t}", name=f"wqT{t}")
                   for t in range(4)]
            wkT = [projp.tile([128, DIM], BF16, tag=f"wkT{t}", name=f"wkT{t}")
                   for t in range(4)]
            wvT = [projp.tile([128, DIM], BF16, tag=f"wvT{t}", name=f"wvT{t}")
                   for t in range(4)]
            for t in range(4):
                nc.sync.dma_start(inpT[t][:], d["inpT"][128 * t:128 * t + 128, :])
                nc.sync.dma_start(wqT[t][:], d["wqT"][128 * t:128 * t + 128, :])
                nc.sync.dma_start(wkT[t][:], d["wkT"][128 * t:128 * t + 128, :])
                nc.sync.dma_start(ctxT[t][:], d["ctxT"][128 * t:128 * t + 128, :])
                nc.sync.dma_start(wvT[t][:], d["wvT"][128 * t:128 * t + 128, :])

            def proj_q(t):
                for c in range(CH):     # i chunk
                    ps = ppd.tile([128, 2 * CW], F32, tag="pdE", name="pdE")[:, 0:CW]
                    for k in range(4):
                        nc.tensor.matmul(ps[:], wqT[k][:, 128 * t:128 * t + 128],
                                         inpT[k][:, CW * c:CW * c + CW],
                                         start=(k == 0), stop=(k == 3))
                    nc.vector.tensor_scalar(QT[t][:, CW * c:CW * c + CW], ps[:],
                                            bq[t][:], None, ALU.add)

            def proj_k(t):
                for c in range(4):      # 2N = 4 chunks
                    ps = ppd.tile([128, 2 * CW], F32, tag="pdE", name="pdE")[:, 0:CW]
                    for k in range(4):
                        nc.tensor.matmul(ps[:], wkT[k][:, 128 * t:128 * t + 128],
                                         ctxT[k][:, CW * c:CW * c + CW],
                                         start=(k == 0), stop=(k == 3))
                    nc.vector.tensor_scalar(KT[t][:, CW * c:CW * c + CW], ps[:],
                                            bk[t][:], None, ALU.add)

            def proj_sk(t):
                for c in range(CH):
                    ps = ppd.tile([128, 2 * CW], F32, tag="pdE", name="pdE")[:, 0:CW]
                    for k in range(4):
                        nc.tensor.matmul(ps[:], wkT[k][:, 128 * t:128 * t + 128],
                                         inpT[k][:, CW * c:CW * c + CW],
                                         start=(k == 0), stop=(k == 3))
                    nc.vector.tensor_scalar(SkT[t][:, CW * c:CW * c + CW], ps[:],
                                            bk[t][:], None, ALU.add)

            # tile 0 of Q/K/Sk first so head-0 dots+exp can start after ~32
            # matmuls; the hoisted dots keep ACT busy through the rest of
            # the projections and the Sv/G phases.
            proj_q(0); proj_k(0); proj_sk(0)
            E_h0, E2_h0 = dots_phase(0)
            for t in range(1, 4):
                proj_q(t)
            for t in range(1, 4):
                proj_k(t)
            for t in range(1, 4):
                proj_sk(t)

            # Sv natural [i, dv]: lhsT = inpT (c -> i), rhs = wvT (c -> dv)
            for t in range(8):          # i tile
                ps = ppd.tile([128, 2 * CW], F32, tag="pdE", name="pdE")[:, 0:CW]
                for k in range(4):
                    nc.tensor.matmul(ps[:], inpT[k][:, 128 * t:128 * t + 128],
                                     wvT[k][:], start=(k == 0), stop=(k == 3))
                # value cols (strided by 65) = psum + bias_bcast; ones cols = 1
                vcols = Sv[t][:].rearrange("p (h c) -> p h c", h=8)[:, :, 0:64]
                nc.vector.tensor_tensor(vcols, ps[:], bsv[:], ALU.add)
                ocols = Sv[t][:].rearrange("p (h c) -> p h c", h=8)[:, :, 64:65]
                nc.vector.memset(ocols, 1.0)

        with tc.tile_pool(name="projq", bufs=1) as projq:
            wfN = [projq.tile([128 if t < 8 else M, NK], BF16,
                              tag=f"wfN{t}", name=f"wfN{t}") for t in range(NFT)]
            for t in range(NFT):
                r0 = 128 * t
                r1 = min(r0 + 128, NF)
                nc.sync.dma_start(wfN[t][:], d["wfN"][r0:r1, :])
            # G = Wf.T @ Sv (all heads at once): G[j, (h,d)] strided like Sv.
            for t in range(NKT):        # j tile
                mw = 128 if t < 16 else M
                ps = ppd.tile([128, 2 * CW], F32, tag="pdE", name="pdE")[:, 0:CW]
                for k in range(NFT):
                    kw = 128 if k < 8 else M
                    rhs = (Sv[k] if k < 8 else svB)[0:kw].rearrange(
                        "p (h c) -> p h c", h=8)[:, :, 0:64]
                    nc.tensor.matmul(ps[0:mw, :],
                                     wfN[k][0:kw, 128 * t:128 * t + mw],
                                     rhs, start=(k == 0), stop=(k == NFT - 1))
                gv = Gst[t][0:mw].rearrange("p (h c) -> p h c", h=8)[:, :, 0:64]
                nc.vector.tensor_copy(gv, ps[0:mw, :])
                go = Gst[t][0:mw].rearrange("p (h c) -> p h c", h=8)[:, :, 64:65]
                nc.vector.memset(go, 1.0)

        # ---------------- attention ----------------
        u_phase(0, E_h0, E2_h0)
        for h in range(1, HEADS):
            E_h, E2_h = dots_phase(h)
            u_phase(h, E_h, E2_h)

        # ---------------- bias terms: w = bf @ Sv per head ----------------
        for h in range(HEADS):
            ht, hp = divmod(h, 2)
            hp *= 64
            pw = ppu.tile([65, CW], F32, tag="pu1", name="pu1")[0:64]
            for t in range(NFT):
                kw = 128 if t < 8 else M
                lhs = (Sv[t] if t < 8 else svB)[0:kw, 65 * h:65 * h + 64]
                nc.tensor.matmul(pw[:, 0:1], lhs, bfc[t][0:kw, :],
                                 start=(t == 0), stop=(t == NFT - 1))
            nc.vector.tensor_copy(wT[ht][hp:hp + 64, :], pw[:, 0:1])
        prow = ppu.tile([65, CW], F32, tag="pu1", name="pu1")[0:64]
        for k in range(4):
            nc.tensor.matmul(prow[0:1, :], wT[k][:], woT[k][:],
                             start=(k == 0), stop=(k == 3))
        nc.vector.tensor_tensor(wob[:], prow[0:1, :], borow[:], ALU.add)

        # ---------------- final projection ----------------
        for t in range(8):
            ps = ppd.tile([128, 2 * CW], F32, tag="pdE", name="pdE")[:, 0:CW]
            for k in range(4):
                nc.tensor.matmul(ps[:], outT[k][:, 128 * t:128 * t + 128],
                                 woT[k][:], start=(k == 0), stop=False)
            nc.tensor.matmul(ps[:], ones128[:], wob[:], start=False, stop=True)
            o_sb = work.tile([128, CW], F32, tag="osb", name="osb")
            nc.vector.tensor_copy(o_sb[:], ps[:])
            nc.sync.dma_start(out_d[128 * t:128 * t + 128, :], o_sb[:])


# ---------------------------------------------------------------------------
# host side
# ---------------------------------------------------------------------------
_CACHE = {}


def _get_nc():
    if "nc" not in _CACHE:
        nc = bacc.Bacc("TRN2", target_bir_lowering=False, debug=False,
                       enable_asserts=False, num_devices=B)
        with tile.TileContext(nc) as tc:
            build_kernel(tc)
        nc.compile()
        _CACHE["nc"] = nc
    return _CACHE["nc"]


def _prep_shared(Wq, bq, Wkv, bkv, Wf, bf, Wo, bo, m_k, m_v, Sm_k, Sm_v):
    f = np.float32
    s = {}
    s["wqT"] = np.ascontiguousarray(np.asarray(Wq, f).T).astype(bfnp)
    s["wkT"] = np.ascontiguousarray(np.asarray(Wkv, f)[:DIM].T).astype(bfnp)
    s["wvT"] = np.ascontiguousarray(np.asarray(Wkv, f)[DIM:].T).astype(bfnp)
    s["woT"] = np.ascontiguousarray(np.asarray(Wo, f).T).astype(bfnp)

    s["wfN"] = np.ascontiguousarray(np.asarray(Wf, f)).astype(bfnp)
    bfv = np.asarray(bf, f)

    mkv = (np.sqrt(DH) * np.broadcast_to(np.asarray(m_k, f), (1, M, DIM))
           ).reshape(HEADS, M, DH)
    smk = (np.sqrt(DH) * np.broadcast_to(np.asarray(Sm_k, f), (1, M, DIM))
           ).reshape(HEADS, M, DH)
    smv = (np.sqrt(M) * np.broadcast_to(np.asarray(Sm_v, f), (1, M, DIM))
           ).reshape(HEADS, M, DH)
    s["memKT"] = np.ascontiguousarray(
        mkv.transpose(0, 2, 1).reshape(DIM, M)).astype(bfnp)
    s["memSkT"] = np.ascontiguousarray(
        smk.transpose(0, 2, 1).reshape(DIM, M)).astype(bfnp)

    svB = np.zeros((M, 520), f)
    for h in range(HEADS):
        svB[:, 65 * h:65 * h + DH] = smv[h]
        svB[:, 65 * h + DH] = 1.0
    s["svmemB"] = svB.astype(bfnp)

    bkv_v = np.asarray(bkv, f)[DIM:]
    s["b_sv"] = np.broadcast_to(bkv_v[None, :], (128, DIM)).astype(bfnp).copy()
    s["bq"] = np.asarray(bq, f).reshape(DIM, 1).copy()
    s["bk"] = np.asarray(bkv, f)[:DIM].reshape(DIM, 1).copy()
    s["bfcol"] = bfv.reshape(NF, 1).astype(bfnp)
    s["borow"] = np.asarray(bo, f).reshape(1, DIM).astype(bfnp)
    return s


def kernel(inp, x, y, Wq, bq, Wkv, bkv, Wf, bf, Wo, bo, m_k, m_v, Sm_k, Sm_v,
           _trace=False):
    f = np.float32
    nc = _get_nc()
    shared = _prep_shared(Wq, bq, Wkv, bkv, Wf, bf, Wo, bo, m_k, m_v, Sm_k, Sm_v)
    inp = np.asarray(inp, f)
    x = np.asarray(x, f)
    y = np.asarray(y, f)
    in_maps = []
    for b in range(B):
        m = dict(shared)
        m["inpT"] = np.ascontiguousarray(inp[b].T).astype(bfnp)
        m["ctxT"] = np.ascontiguousarray(
            np.concatenate([x[b], y[b]], 0).T).astype(bfnp)
        in_maps.append(m)
    res = bass_utils.run_bass_kernel_spmd(
        nc, in_maps, core_ids=list(range(B)),
        **({"trace": True, "trace_cores": [0]} if _trace else {}))
    out = np.stack([np.asarray(res.results[b]["out"]) for b in range(B)], 0)
    if _trace:
        _CACHE["last_results"] = res
    return out



# revision 58
# speedup vs baseline: 1.2193x; 1.2193x over previous
"""CMAttention Trainium2 kernel (Bass/Tile), data-parallel over batch on 8 cores.

Reference computation (per batch b, per head h, d=64, n=1024):
  q = inp @ Wq.T + bq                    (split heads)
  k = [ctx @ Wk.T + bk ; sqrt(64)*mk]    ctx = [x;y], 2051 keys
  cross = softmax(q k^T / 8) @ Wf.T + bf          [n, 1027]
  Sk = [inp @ Wk.T + bk ; sqrt(64)*smk]  1027 self keys
  self = softmax(q Sk^T / 8)                       [n, 1027]
  Sv = [inp @ Wv.T + bv ; sqrt(3)*smv]             [1027, 64]
  out_h = (cross + self) @ Sv
  out = concat_h(out_h) @ Wo.T + bo

Kernel strategy (v2; the HW cost model charges a matmul out_free_rows only):
  - bf16 matmuls with fp32 PSUM accumulation; unnormalized softmax
    E = exp(dots*scale), normalization deferred past both E-contractions.
  - Fuse trick: (E @ Wf.T) @ Sv == E @ G with G = Wf.T @ Sv  [2051, 520],
    carrying a ones column per head so PSUM col 64 is the softmax
    denominator Z.
  - dots^T[j, i] per head via matmul(lhsT=KT, rhs=QT): 16 full j-tiles
    (cross) + 8 (self); one [128,1024] exp per tile on ACT (ACT is the
    bottleneck: ~200us of exp is irreducible here).
  - U phase REORIENTED vs v1: out[i-tile(128 partitions), 65 free] with
    lhsT=E-tile column block, rhs=G tile head block. Full partition use,
    65-row instructions: ~108k PE rows vs 213k for the old [65, 1024]
    orientation.  Normalization becomes a per-partition reciprocal column
    + tensor_scalar; the two branches fuse via scalar_tensor_tensor.
  - Memory-token dots packed across ALL heads: block-diag [d, 6] mem-key
    tiles per head pair -> one [48, 1024] psum -> ONE exp (replaces 16
    separate 3-row exps that each cost a full instruction).
  - Ub [1024, 512] transposed to UbT via dma_start_transpose (idle DMA
    engines, zero compute-engine cost), spread across the U rounds; final
    projection contracts UbT against woT; output bias folded into the
    psum->sbuf copy (tensor_tensor add with a partition-broadcast tile).
  - G = Wf.T @ Sv computed in two passes of k-chunks (5 then 4) so only 5
    wfN row-tiles are SBUF-resident; full-f32 accumulation for the mem
    j-tile via a dedicated psum bank that survives both passes.
  - Engine order: proj, dots_h0, memdots, G, dots_h1_cross, then rounds
    [U_h, dots_{h+1}self, dots_{h+2}cross].  Cross-E pool double buffered
    (so exp of head h+2 never waits on U_h), self-E single buffered
    (WAR-safe by construction).  ACT stays saturated throughout.
  - All biases (zero in this problem's setup_inputs, but implemented
    faithfully) ride the projections' tensor_scalar adds, the bf@Sv@Wo
    ones-trick row, and the fused output bias add.
"""
import numpy as np
import ml_dtypes
from contextlib import ExitStack

import concourse.bass as bass
import concourse.tile as tile
from concourse import bacc, mybir
from concourse import bass_utils

F32 = mybir.dt.float32
BF16 = mybir.dt.bfloat16
bfnp = ml_dtypes.bfloat16
AF = mybir.ActivationFunctionType
ALU = mybir.AluOpType

B = 8
N = 1024
DIM = 512
HEADS = 8
DH = 64
M = 3
SCALE = 0.125
NK = 2 * N + M          # 2051 cross keys
NF = N + M              # 1027 fuse outputs / self keys
NCT = 16                # full cross j-tiles (2048 = 16*128)
NST = 8                 # full self j-tiles (1024 = 8*128)
CH = 2                  # i-chunks of 512
CW = 512
GKA = 5                 # G pass-A k-chunks (wfN rows 0..639)
GKB = 4                 # G pass-B k-chunks (wfN rows 640..1026)


def build_kernel(tc):
    nc = tc.nc
    d = {}
    def din(name, shape, dt=BF16):
        d[name] = nc.dram_tensor(name, shape, dt, kind="ExternalInput").ap()
    din("inpT", [DIM, N])
    din("ctxT", [DIM, 2 * N])
    din("wqT", [DIM, DIM])
    din("wkT", [DIM, DIM])
    din("wvT", [DIM, DIM])
    din("woT", [DIM, DIM])
    din("wfN", [NF, NK])         # Wf natural [1027, 2051]
    din("bdK", [DIM, 24])        # zero-padded block-diag mem keys
    din("bdSk", [DIM, 24])
    din("svB", [M, 520])         # smv rows + ones cols
    din("svBpad", [24, 520])     # per-head masked smv replicas
    din("b_sv", [128, DIM])      # bkv_v broadcast tile
    din("bq", [DIM, 1], F32)
    din("bk", [DIM, 1], F32)
    din("bfcol", [NF, 1])        # bf
    din("borow", [1, DIM])
    din("ident", [128, 128])
    out_d = nc.dram_tensor("out", [N, DIM], F32, kind="ExternalOutput").ap()

    def hview(t):
        # [p, 520] -> [p, 8, 64] value columns (skip per-head ones col)
        return t.rearrange("p (h c) -> p h c", h=8)[:, :, 0:64]

    ctx = ExitStack()
    with ctx:
        pers = ctx.enter_context(tc.tile_pool(name="pers", bufs=1))
        ppd = ctx.enter_context(tc.tile_pool(name="ppd", bufs=2, space="PSUM"))
        ppx = ctx.enter_context(tc.tile_pool(name="ppx", bufs=1, space="PSUM"))
        _pxc = [0]

        def px_tile():
            # alternating 1-bank psum tiles for DVE-consumed results
            # (projections, G columns) -- keeps them out of the
            # exp-consumed pdE ring so ACT never stalls behind DVE
            tag = f"px{_pxc[0] & 1}"
            _pxc[0] += 1
            return ppx.tile([128, CW], F32, tag=tag, name=tag)

        # ---------------- persistent SBUF ----------------
        woT = [pers.tile([128, DIM], BF16, tag=f"woT{t}", name=f"woT{t}") for t in range(4)]
        Gst = [pers.tile([128, 520], BF16, tag=f"Gst{t}", name=f"Gst{t}")
               for t in range(NCT)]
        QT = [pers.tile([128, N], BF16, tag=f"QT{t}", name=f"QT{t}") for t in range(4)]
        KT = [pers.tile([128, 2 * N], BF16, tag=f"KT{t}", name=f"KT{t}") for t in range(4)]
        SkT = [pers.tile([128, N], BF16, tag=f"SkT{t}", name=f"SkT{t}") for t in range(4)]
        Sv = [pers.tile([128, 520], BF16, tag=f"Sv{t}", name=f"Sv{t}") for t in range(8)]
        svB = pers.tile([M, 520], BF16, tag="svB", name="svB")
        svBp = pers.tile([56, 520], BF16, tag="svBp", name="svBp")
        Gpad = pers.tile([24, 520], BF16, tag="Gpad", name="Gpad")
        Emem = pers.tile([56, N], BF16, tag="Emem", name="Emem")
        bfc = [pers.tile([128 if t < 8 else M, 1], BF16, tag=f"bfc{t}", name=f"bfc{t}")
               for t in range(9)]
        wT = [pers.tile([128, 1], BF16, tag=f"wT{t}", name=f"wT{t}") for t in range(4)]
        wobB = pers.tile([128, DIM], BF16, tag="wobB", name="wobB")
        wob = pers.tile([1, DIM], BF16, tag="wob", name="wob")
        borow = svBp[0:1, 0:DIM]   # svBp rows 0..32 are otherwise unused
        idt = pers.tile([128, 128], BF16, tag="idt", name="idt")

        # E pools: cross double-buffered (so head h+2's exps never wait on
        # U_h), self single-buffered (safe: self_{h+1} dots are emitted
        # after U_h on PE)
        ecp = ctx.enter_context(tc.tile_pool(name="ecp", bufs=2))
        esp = ctx.enter_context(tc.tile_pool(name="esp", bufs=1))
        work = ctx.enter_context(tc.tile_pool(name="work", bufs=2))
        wfp = None  # bound in the preamble scope; used by wf_wave below

        def wf_wave(ks):
            """DMA wfN row-chunks ks (3 of them) into the ring (Pool queue:
            its SWDGE engine is otherwise idle and never blocks compute)."""
            wf = {}
            for k in ks:
                r0 = 128 * k
                r1 = min(r0 + 128, NF)
                w = wfp.tile([128, NK], BF16, tag=f"wfN{k % 3}",
                             name=f"wfN{k % 3}")
                wf[k] = w
                nc.gpsimd.dma_start(w[0:r1 - r0, :], d["wfN"][r0:r1, :])
            return wf

        def g_wave(wf, ks, pr, first, pg16):
            """One contraction wave of G for head-pair pr (columns only).
            pg16 accumulates the mem j-tile for ALL heads (pair 0 only)."""
            for t in range(NCT):
                ps = ppd.tile([128, 2 * CW], F32, tag="pdE", name="pdE")[:, 0:128]
                for i, k in enumerate(ks):
                    kw = 128 if k < 8 else M
                    rhs = (hview(Sv[k][0:kw]) if k < 8
                           else hview(svB[0:M]))[:, 2 * pr:2 * pr + 2, :]
                    nc.tensor.matmul(ps[:], wf[k][0:kw, 128 * t:128 * t + 128],
                                     rhs, start=(i == 0),
                                     stop=(i == len(ks) - 1))
                gv = hview(Gst[t][:])[:, 2 * pr:2 * pr + 2, :]
                if first:
                    nc.vector.tensor_copy(gv, ps[:])
                else:
                    nc.vector.tensor_tensor(gv, ps[:], gv, ALU.add)
                if ks[-1] == 8:  # last wave: set this pair's ones columns
                    go = Gst[t][:].rearrange("p (h c) -> p h c",
                                             h=8)[:, 2 * pr:2 * pr + 2, 64:65]
                    nc.vector.memset(go, 1.0)
            if pg16 is not None:
                for i, k in enumerate(ks):
                    kw = 128 if k < 8 else M
                    rhs = hview(Sv[k][0:kw]) if k < 8 else hview(svB[0:M])
                    nc.tensor.matmul(pg16[:], wf[k][0:kw, 2 * N:2 * N + M],
                                     rhs, start=(first and i == 0),
                                     stop=(k == 8))

        def cross_tile(h, t, E):
            ht, sd = divmod(h, 2)
            hp = 64 * sd
            ps = ppd.tile([128, 2 * CW], F32, tag="pdE", name="pdE")
            for c in range(CH):
                nc.tensor.matmul(ps[:, CW * c:CW * c + CW],
                                 KT[ht][hp:hp + 64, 128 * t:128 * t + 128],
                                 QT[ht][hp:hp + 64, CW * c:CW * c + CW],
                                 start=True, stop=True)
            e = ecp.tile([128, 2 * CW], BF16, tag=f"E{t}", name=f"E{t}")
            E[t] = e
            nc.scalar.activation(e[:], ps[:], AF.Exp, scale=SCALE)

        def self_tile(h, t, E2):
            ht, sd = divmod(h, 2)
            hp = 64 * sd
            ps = ppd.tile([128, 2 * CW], F32, tag="pdE", name="pdE")
            for c in range(CH):
                nc.tensor.matmul(ps[:, CW * c:CW * c + CW],
                                 SkT[ht][hp:hp + 64, 128 * t:128 * t + 128],
                                 QT[ht][hp:hp + 64, CW * c:CW * c + CW],
                                 start=True, stop=True)
            e = esp.tile([128, 2 * CW], BF16, tag=f"F{t}", name=f"F{t}")
            E2[t] = e
            nc.scalar.activation(e[:], ps[:], AF.Exp, scale=SCALE)

        def dots_self(h):
            E2 = {}
            for t in range(NST):
                self_tile(h, t, E2)
            return E2

        # =========== preamble: projections, heads 0/1 cross, G pair 0 =====
        with tc.tile_pool(name="projA", bufs=1) as projA, \
             tc.tile_pool(name="wfp", bufs=1) as wfp:
            # wfN ring for G pair 0: 3 row-tiles, reloaded per 3-chunk wave
            inpT = [projA.tile([128, N], BF16, tag=f"inpT{t}", name=f"inpT{t}")
                    for t in range(4)]
            wkT = [projA.tile([128, DIM], BF16, tag=f"wkT{t}", name=f"wkT{t}")
                   for t in range(4)]
            ctxT = [projA.tile([128, 2 * N], BF16, tag=f"ctxT{t}", name=f"ctxT{t}")
                    for t in range(4)]
            wqT = [projA.tile([128, DIM], BF16, tag=f"wqT{t}", name=f"wqT{t}")
                   for t in range(4)]
            wvT = [projA.tile([128, DIM], BF16, tag=f"wvT{t}", name=f"wvT{t}")
                   for t in range(4)]
            bdKt = [projA.tile([128, 24], BF16, tag=f"bdK{t}", name=f"bdK{t}")
                    for t in range(4)]
            bdSt = [projA.tile([128, 24], BF16, tag=f"bdS{t}", name=f"bdS{t}")
                    for t in range(4)]
            bsv = projA.tile([128, DIM], BF16, tag="bsv", name="bsv")
            Gmem = projA.tile([M, 520], BF16, tag="Gmem", name="Gmem")
            bq = [projA.tile([128, 1], F32, tag=f"bq{t}", name=f"bq{t}")
                  for t in range(4)]
            bk = [projA.tile([128, 1], F32, tag=f"bk{t}", name=f"bk{t}")
                  for t in range(4)]
            # DMA order tuned so q0 -> k0 -> dots_h0 starts earliest
            for t in range(4):
                nc.sync.dma_start(inpT[t][:], d["inpT"][128 * t:128 * t + 128, :])
            for t in range(4):
                nc.scalar.dma_start(wqT[t][:], d["wqT"][128 * t:128 * t + 128, :])
            for t in range(4):
                nc.scalar.dma_start(wkT[t][:], d["wkT"][128 * t:128 * t + 128, :])
            for t in range(2):
                nc.sync.dma_start(ctxT[t][:], d["ctxT"][128 * t:128 * t + 128, :])
                nc.scalar.dma_start(ctxT[2 + t][:],
                                    d["ctxT"][128 * (2 + t):128 * (2 + t) + 128, :])
            for t in range(4):
                nc.sync.dma_start(bq[t][:], d["bq"][128 * t:128 * t + 128, :])
                nc.sync.dma_start(bk[t][:], d["bk"][128 * t:128 * t + 128, :])
            nc.sync.dma_start(bsv[:], d["b_sv"][:])
            for t in range(4):
                nc.sync.dma_start(wvT[t][:], d["wvT"][128 * t:128 * t + 128, :])
            wfA = wf_wave([0, 1, 2])

            def proj_q_c(t, c):
                ps = px_tile()
                for k in range(4):
                    nc.tensor.matmul(ps[:], wqT[k][:, 128 * t:128 * t + 128],
                                     inpT[k][:, CW * c:CW * c + CW],
                                     start=(k == 0), stop=(k == 3))
                nc.vector.tensor_scalar(QT[t][:, CW * c:CW * c + CW], ps[:],
                                        bq[t][:], None, ALU.add)

            def proj_k_c(t, c):
                ps = px_tile()
                for k in range(4):
                    nc.tensor.matmul(ps[:], wkT[k][:, 128 * t:128 * t + 128],
                                     ctxT[k][:, CW * c:CW * c + CW],
                                     start=(k == 0), stop=(k == 3))
                nc.vector.tensor_scalar(KT[t][:, CW * c:CW * c + CW], ps[:],
                                        bk[t][:], None, ALU.add)

            def proj_sk_c(t, c):
                ps = px_tile()
                for k in range(4):
                    nc.tensor.matmul(ps[:], wkT[k][:, 128 * t:128 * t + 128],
                                     inpT[k][:, CW * c:CW * c + CW],
                                     start=(k == 0), stop=(k == 3))
                nc.vector.tensor_scalar(SkT[t][:, CW * c:CW * c + CW], ps[:],
                                        bk[t][:], None, ALU.add)

            def proj_sv_t(t):
                ps = px_tile()
                for k in range(4):
                    nc.tensor.matmul(ps[:], inpT[k][:, 128 * t:128 * t + 128],
                                     wvT[k][:], start=(k == 0), stop=(k == 3))
                nc.vector.tensor_tensor(hview(Sv[t][:]), ps[:], bsv[:],
                                        ALU.add)
                ones = Sv[t][:].rearrange("p (h c) -> p h c", h=8)[:, :, 64:65]
                nc.vector.memset(ones, 1.0)

            # q0/k0 ahead of everything: dots_h0 wants them
            proj_q_c(0, 0); proj_q_c(0, 1)
            for c in range(4):
                proj_k_c(0, c)

            # non-urgent persistent DMAs (scalar queue, after exp dispatches)
            for t in range(4):
                nc.scalar.dma_start(woT[t][:], d["woT"][128 * t:128 * t + 128, :])
                nc.scalar.dma_start(bdKt[t][:], d["bdK"][128 * t:128 * t + 128, :])
                nc.scalar.dma_start(bdSt[t][:], d["bdSk"][128 * t:128 * t + 128, :])
            for t in range(9):
                r0 = 128 * t
                r1 = min(r0 + 128, NF)
                nc.scalar.dma_start(bfc[t][:], d["bfcol"][r0:r1, :])
            nc.scalar.dma_start(svB[:], d["svB"][:])
            nc.scalar.dma_start(svBp[32:56, :], d["svBpad"][:])
            nc.scalar.dma_start(borow, d["borow"][:])
            nc.scalar.dma_start(idt[:], d["ident"][:])
            nc.vector.memset(Gpad[:], 0.0)

            with tc.tile_pool(name="pgp", bufs=1, space="PSUM") as pgp:
                pg16 = pgp.tile([M, CW], F32, tag="pg16", name="pg16")
                wf = {}

                def g_tile(ks, t, first):
                    ps = px_tile()[:, 0:128]
                    for i, k in enumerate(ks):
                        kw = 128 if k < 8 else M
                        rhs = (hview(Sv[k][0:kw]) if k < 8
                               else hview(svB[0:M]))[:, 0:2, :]
                        nc.tensor.matmul(ps[:],
                                         wf[k][0:kw, 128 * t:128 * t + 128],
                                         rhs, start=(i == 0),
                                         stop=(i == len(ks) - 1))
                    gv = hview(Gst[t][:])[:, 0:2, :]
                    if first:
                        nc.vector.tensor_copy(gv, ps[:])
                    else:
                        nc.vector.tensor_tensor(gv, ps[:], gv, ALU.add)
                    if ks[-1] == 8:
                        go = Gst[t][:].rearrange("p (h c) -> p h c",
                                                 h=8)[:, 0:2, 64:65]
                        nc.vector.memset(go, 1.0)

                def g_pg16(ks, first):
                    for i, k in enumerate(ks):
                        kw = 128 if k < 8 else M
                        rhs = hview(Sv[k][0:kw]) if k < 8 else hview(svB[0:M])
                        nc.tensor.matmul(pg16[:],
                                         wf[k][0:kw, 2 * N:2 * N + M], rhs,
                                         start=(first and i == 0),
                                         stop=(k == 8))

                def memdots(sel):
                    # rows 3h+r = cross mem r of head h; +32 = self branch
                    # (psum base partition must be 0/32/64); exp-consumed,
                    # so the pdE ring is the right home
                    if sel == 0:
                        memdots.pmm = ppd.tile([128, 2 * CW], F32, tag="pdE",
                                               name="pdE")
                    pmm = memdots.pmm
                    r0, bd = (slice(0, 24), bdKt) if sel == 0 \
                        else (slice(32, 56), bdSt)
                    for c in range(CH):
                        cs = slice(CW * c, CW * c + CW)
                        for t in range(4):
                            nc.tensor.matmul(pmm[r0, cs], bd[t][:, :],
                                             QT[t][:, cs],
                                             start=(t == 0), stop=(t == 3))
                    er = slice(0, 24) if sel == 0 else slice(32, 56)
                    nc.scalar.activation(Emem[er, :], pmm[er, :], AF.Exp,
                                         scale=SCALE)

                def g_finish():
                    nc.vector.tensor_copy(hview(Gmem[:]), pg16[:])
                    gones = Gmem[:].rearrange("p (h c) -> p h c",
                                              h=8)[:, :, 64:65]
                    nc.vector.memset(gones, 1.0)
                    for h in range(HEADS):
                        nc.sync.dma_start(
                            Gpad[3 * h:3 * h + 3, 65 * h:65 * h + 65],
                            Gmem[0:M, 65 * h:65 * h + 65])

                def bias_w():
                    for h in range(HEADS):
                        ht, hp = divmod(h, 2)
                        hp *= 64
                        pw = pgp.tile([65, CW], F32, tag="pwb", name="pwb")
                        for t in range(9):
                            kw = 128 if t < 8 else M
                            lhs = (Sv[t][0:kw, 65 * h:65 * h + 64] if t < 8
                                   else svB[0:M, 65 * h:65 * h + 64])
                            nc.tensor.matmul(pw[0:64, 0:1], lhs,
                                             bfc[t][0:kw, :],
                                             start=(t == 0), stop=(t == 8))
                        nc.vector.tensor_copy(wT[ht][hp:hp + 64, :],
                                              pw[0:64, 0:1])
                    prow = pgp.tile([65, CW], F32, tag="pwb", name="pwb")
                    for k in range(4):
                        nc.tensor.matmul(prow[0:1, :], wT[k][:], woT[k][:],
                                         start=(k == 0), stop=(k == 3))
                    nc.vector.tensor_tensor(wob[:], prow[0:1, :], borow,
                                            ALU.add)
                    nc.gpsimd.partition_broadcast(wobB[:], wob[:])

                # ---- interleave: one filler unit (~1us of non-exp PE
                # work) per dots tile, so ACT chews exps while PE does the
                # projections / G / memdots, instead of big serial phases
                FA, FB, FC = [0, 1, 2], [3, 4, 5], [6, 7, 8]
                fillers = (
                    [lambda c=c: proj_sk_c(0, c) for c in range(CH)] +
                    [lambda t=t: proj_sv_t(t) for t in range(8)] +
                    [lambda t=t, c=c: proj_q_c(t, c)
                     for t in range(1, 4) for c in range(CH)] +
                    [lambda ts=ts: [g_tile(FA, t, True) for t in ts]
                     for ts in (range(0, 6), range(6, 12), range(12, 16))] +
                    [lambda: (g_pg16(FA, True), wf.update(wf_wave(FB)))] +
                    [lambda c=c: proj_k_c(1, c) for c in range(4)] +
                    [lambda ts=ts: [g_tile(FB, t, False) for t in ts]
                     for ts in (range(0, 6), range(6, 12), range(12, 16))] +
                    [lambda: (g_pg16(FB, False), wf.update(wf_wave(FC)))] +
                    [lambda c=c: proj_k_c(2, c) for c in range(4)] +
                    [lambda ts=ts: [g_tile(FC, t, False) for t in ts]
                     for ts in (range(0, 6), range(6, 12), range(12, 16))] +
                    [lambda: (g_pg16(FC, False), g_finish())] +
                    [lambda c=c: proj_k_c(3, c) for c in range(4)] +
                    [lambda t=t, c=c: proj_sk_c(t, c)
                     for t in range(1, 4) for c in range(CH)] +
                    [lambda: memdots(0), lambda: memdots(1)] +
                    [bias_w]
                )
                Ec = {0: {}, 1: {}}
                Es = {0: {}}
                dots = (
                    [lambda t=t: cross_tile(0, t, Ec[0]) for t in range(NCT)] +
                    [lambda t=t: self_tile(0, t, Es[0]) for t in range(NST)] +
                    [lambda t=t: cross_tile(1, t, Ec[1]) for t in range(NCT)]
                )
                wf.update(wfA)
                di = fi = 0
                while di < len(dots) or fi < len(fillers):
                    if di < len(dots):
                        dots[di]()
                        di += 1
                    if fi < len(fillers):
                        fillers[fi]()
                        fi += 1

        # =========== attention rounds (ACT-paced at ~25us/head) ===========
        with tc.tile_pool(name="wfb", bufs=1) as wfb, \
             tc.tile_pool(name="late", bufs=1) as late:
            pup = tc.alloc_tile_pool(name="pup", bufs=2, space="PSUM")
            # full wfN, loaded ONCE into the space the projections freed;
            # rounds' G pairs then run with zero DMA stalls
            wfF = [wfb.tile([128, NK], BF16, tag=f"wfF{k}", name=f"wfF{k}")
                   for k in range(9)]
            for k in range(9):
                r0 = 128 * k
                r1 = min(r0 + 128, NF)
                nc.gpsimd.dma_start(wfF[k][0:r1 - r0, :], d["wfN"][r0:r1, :])

            def g_pair_tile(pr, t):
                # full-contraction G for head-pair pr, j-tile t (psum on
                # the DVE-consumed px ring, never the exp ring)
                ps = px_tile()[:, 0:128]
                for k in range(9):
                    kw = 128 if k < 8 else M
                    rhs = (hview(Sv[k][0:kw]) if k < 8
                           else hview(svB[0:M]))[:, 2 * pr:2 * pr + 2, :]
                    nc.tensor.matmul(ps[:],
                                     wfF[k][0:kw, 128 * t:128 * t + 128],
                                     rhs, start=(k == 0), stop=(k == 8))
                gv = hview(Gst[t][:])[:, 2 * pr:2 * pr + 2, :]
                nc.vector.tensor_copy(gv, ps[:])
                go = Gst[t][:].rearrange("p (h c) -> p h c",
                                         h=8)[:, 2 * pr:2 * pr + 2, 64:65]
                nc.vector.memset(go, 1.0)

            Ub = [[late.tile([128, 128], BF16, tag=f"Ub{ct}_{it}",
                             name=f"Ub{ct}_{it}") for it in range(8)]
                  for ct in range(4)]
            UbTt = [[late.tile([128, 128], BF16, tag=f"UbT{ct}_{it}",
                               name=f"UbT{ct}_{it}") for it in range(8)]
                    for ct in range(4)]

            def u_phase(h, E, E2):
                hs = slice(65 * h, 65 * h + 65)
                for it in range(8):
                    isl = slice(128 * it, 128 * it + 128)
                    # both branches share ONE psum bank: U1 in cols 0:65,
                    # U2 in 65:130.  The U1 start=True marks the whole 2KB
                    # bank pending-zero, so U2 accumulates from zero with
                    # start=False (group check skipped: different columns).
                    pu = pup.tile([128, 130], F32, tag="pu", name="pu")
                    for t in range(NCT):
                        nc.tensor.matmul(pu[:, 0:65], E[t][:, isl],
                                         Gst[t][:, hs],
                                         start=(t == 0), stop=False)
                    nc.tensor.matmul(pu[:, 0:65], Emem[0:24, isl],
                                     Gpad[0:24, hs], start=False, stop=True)
                    for t in range(NST):
                        nc.tensor.matmul(pu[:, 65:130], E2[t][:, isl],
                                         Sv[t][:, hs], start=False,
                                         stop=False, skip_group_check=True)
                    nc.tensor.matmul(pu[:, 65:130], Emem[32:56, isl],
                                     svBp[32:56, hs], start=False, stop=True,
                                     skip_group_check=True)
                    # normalize + combine both branches
                    rz1 = work.tile([128, 1], F32, tag="rz1", name="rz1")
                    rz2 = work.tile([128, 1], F32, tag="rz2", name="rz2")
                    nc.vector.reciprocal(rz1[:], pu[:, 64:65])
                    nc.vector.reciprocal(rz2[:], pu[:, 129:130])
                    tmp = work.tile([128, 64], BF16, tag="tmp", name="tmp")
                    nc.vector.tensor_scalar(tmp[:], pu[:, 0:64], rz1[:],
                                            None, ALU.mult)
                    nc.vector.scalar_tensor_tensor(
                        Ub[h // 2][it][:, 64 * (h % 2):64 * (h % 2) + 64],
                        pu[:, 65:129], rz2[:], tmp[:], ALU.mult, ALU.add)
                    if h == 7:
                        # last head: transpose this i-tile's Ub column
                        # block right away so the final projection can
                        # start per-tile (DMA latency hidden behind the
                        # remaining norms)
                        eng = nc.sync if it % 2 == 0 else nc.scalar
                        eng.dma_start_transpose(UbTt[3][it][:], Ub[3][it][:])

            for h in range(HEADS):
                u_phase(h, Ec.pop(h), Es.pop(h))
                if h + 1 < HEADS:
                    Es[h + 1] = dots_self(h + 1)
                if h + 2 < HEADS:
                    # cross dots for head h+2, interleaved with G columns
                    # for pair (h+1)//2 (odd rounds) so exp-feeding psums
                    # keep flowing and G never adjoins the next U on the
                    # Gst write->read dependency chain
                    Ec[h + 2] = E = {}
                    pr = (h + 1) // 2 if h in (1, 3, 5) else None
                    for t in range(NCT):
                        cross_tile(h + 2, t, E)
                        if pr is not None:
                            g_pair_tile(pr, t)
                if h % 2 == 1 and h < 7:
                    # head pair (h-1, h) done: transpose its Ub column block
                    ct = h // 2
                    eng = nc.sync if ct % 2 == 0 else nc.scalar
                    for it in range(8):
                        eng.dma_start_transpose(UbTt[ct][it][:], Ub[ct][it][:])

            pup.release()
            # ------------- tail: final projection ------------------------
            for it in range(8):
                ps = ppd.tile([128, 2 * CW], F32, tag="pdE", name="pdE")[:, 0:CW]
                for k in range(4):
                    nc.tensor.matmul(ps[:], UbTt[k][it][:], woT[k][:],
                                     start=(k == 0), stop=(k == 3))
                o_sb = late.tile([128, CW], F32, tag=f"osb{it % 2}",
                                 name=f"osb{it % 2}")
                nc.vector.tensor_tensor(o_sb[:], ps[:], wobB[:], ALU.add)
                eng = nc.sync if it % 2 == 0 else nc.scalar
                eng.dma_start(out_d[128 * it:128 * it + 128, :], o_sb[:])


# ---------------------------------------------------------------------------
# host side
# ---------------------------------------------------------------------------
_CACHE = {}


def _get_nc():
    if "nc" not in _CACHE:
        nc = bacc.Bacc("TRN2", target_bir_lowering=False, debug=False,
                       enable_asserts=False, num_devices=B)
        with tile.TileContext(nc) as tc:
            build_kernel(tc)
        nc.compile()
        _CACHE["nc"] = nc
    return _CACHE["nc"]


def _prep_shared(Wq, bq, Wkv, bkv, Wf, bf, Wo, bo, m_k, m_v, Sm_k, Sm_v):
    f = np.float32
    s = {}
    s["wqT"] = np.ascontiguousarray(np.asarray(Wq, f).T).astype(bfnp)
    s["wkT"] = np.ascontiguousarray(np.asarray(Wkv, f)[:DIM].T).astype(bfnp)
    s["wvT"] = np.ascontiguousarray(np.asarray(Wkv, f)[DIM:].T).astype(bfnp)
    s["woT"] = np.ascontiguousarray(np.asarray(Wo, f).T).astype(bfnp)
    s["wfN"] = np.ascontiguousarray(np.asarray(Wf, f)).astype(bfnp)

    mkv = (np.sqrt(DH) * np.broadcast_to(np.asarray(m_k, f), (1, M, DIM))
           ).reshape(HEADS, M, DH)
    smk = (np.sqrt(DH) * np.broadcast_to(np.asarray(Sm_k, f), (1, M, DIM))
           ).reshape(HEADS, M, DH)
    smv = (np.sqrt(M) * np.broadcast_to(np.asarray(Sm_v, f), (1, M, DIM))
           ).reshape(HEADS, M, DH)

    # zero-padded block-diag mem-key tiles: rows 128t+64s+d hold head
    # 2t+s's mem keys in columns 3(2t+s)..+3, zeros elsewhere, so the 4
    # per-pair matmuls accumulate all heads at psum base partition 0
    bdK = np.zeros((DIM, 24), f)
    bdS = np.zeros((DIM, 24), f)
    for t in range(4):
        for s_ in range(2):
            h = 2 * t + s_
            r = slice(128 * t + 64 * s_, 128 * t + 64 * s_ + 64)
            bdK[r, 3 * h:3 * h + 3] = mkv[h].T
            bdS[r, 3 * h:3 * h + 3] = smk[h].T
    s["bdK"] = bdK.astype(bfnp)
    s["bdSk"] = bdS.astype(bfnp)

    svB = np.zeros((M, 520), f)
    for h in range(HEADS):
        svB[:, 65 * h:65 * h + DH] = smv[h]
        svB[:, 65 * h + DH] = 1.0
    s["svB"] = svB.astype(bfnp)
    # per-head masked replicas: rows 3h..3h+3 hold ONLY head h's columns
    svp = np.zeros((24, 520), f)
    for h in range(HEADS):
        svp[3 * h:3 * h + 3, 65 * h:65 * h + 65] = svB[:, 65 * h:65 * h + 65]
    s["svBpad"] = svp.astype(bfnp)

    bkv_v = np.asarray(bkv, f)[DIM:]
    s["b_sv"] = np.broadcast_to(bkv_v[None, :], (128, DIM)).astype(bfnp).copy()
    s["bq"] = np.asarray(bq, f).reshape(DIM, 1).copy()
    s["bk"] = np.asarray(bkv, f)[:DIM].reshape(DIM, 1).copy()
    s["bfcol"] = np.asarray(bf, f).reshape(NF, 1).astype(bfnp)
    s["borow"] = np.asarray(bo, f).reshape(1, DIM).astype(bfnp)
    s["ident"] = np.eye(128, dtype=f).astype(bfnp)
    return s


def kernel(inp, x, y, Wq, bq, Wkv, bkv, Wf, bf, Wo, bo, m_k, m_v, Sm_k, Sm_v,
           _trace=False):
    f = np.float32
    nc = _get_nc()
    shared = _prep_shared(Wq, bq, Wkv, bkv, Wf, bf, Wo, bo, m_k, m_v, Sm_k, Sm_v)
    inp = np.asarray(inp, f)
    x = np.asarray(x, f)
    y = np.asarray(y, f)
    in_maps = []
    for b in range(B):
        m = dict(shared)
        m["inpT"] = np.ascontiguousarray(inp[b].T).astype(bfnp)
        m["ctxT"] = np.ascontiguousarray(
            np.concatenate([x[b], y[b]], 0).T).astype(bfnp)
        in_maps.append(m)
    res = bass_utils.run_bass_kernel_spmd(
        nc, in_maps, core_ids=list(range(B)),
        **({"trace": True, "trace_cores": [0]} if _trace else {}))
    out = np.stack([np.asarray(res.results[b]["out"]) for b in range(B)], 0)
    if _trace:
        _CACHE["last_results"] = res
    return out
